# revision 29
# baseline (speedup 1.0000x reference)
"""Sparse masked attention on 8 TRN2 NeuronCores.

reference:  O = softmax((Q K^T * mq[:,None] + log(mk[None,:])) / 8) @ V
  - rows with mq=0: scores all equal -> uniform average of V over mk=1 keys
  - keys with mk=0: exactly dropped from the softmax

Strategy: batch (b=8) is data-parallel across the 8 cores. On the host we
compact each batch to its unmasked queries/keys (~n/2 each), so each core
computes a dense ~2176x2176 attention instead of 4096x4096:

  S^T[mchunk, n] = (Kc^T chunk).T @ Qc^T      (TensorE bf16, d=64, row-group
                                               pairs: two chunks co-stream)
  P^T = exp(S^T / 8) -> bf16                  (ScalarE exact exp on chunk A,
                                               VectorE fast-exp on chunk B,
                                               concurrently each slot)
  O^T[65, n]    += Vext[mchunk].T @ P^T       (TensorE bf16, k=64 row groups
                                               into olo/ohi PSUM accumulators)

where Vext = [V | 1]: the ones column accumulates the softmax denominator.
No row-max subtraction is needed: scores/8 ~ N(0,1), exp stays in range.
Every matmul keeps PE tile_size (64,128) so the array never pays a
reconfiguration bubble; S matmuls are issued one pair-slot ahead of the
exp-gated PV matmuls so the two exp engines never serialize against the
in-order PE queue. The host divides by the denominator, scatters rows
back, and fills masked query rows with mean(V[mk=1]).
"""

import numpy as np
import ml_dtypes

N_CORES = 8
W = 512  # n-block width (PSUM bank / fp32-accum matmul free-dim limit)


_build_cache = {}


def _blocks(ncap):
    """n-blocks as (col_offset, width); the remainder block runs last (it
    needs all K/V chunks, which are still streaming in at kernel start)."""
    blks = [(i * W, W) for i in range(ncap // W)]
    if ncap % W:
        blks.append((ncap - ncap % W, ncap % W))
    return blks


def _build(ncap, mcap):
    """Per-core graph. Inputs (per core):
      qt   [64, ncap]            bf16   Q^T (compacted, padded)
      ktp  [128, npairs*128]     bf16   K^T chunk pairs: pair p = chunk 2p on
                                        partitions 0-63, chunk 2p+1 on 64-127
      vext [128, mchunks*65]     bf16   partition-major Vext chunks: partition
                                        r, cols [c*65:(c+1)*65] = Vext row
                                        c*128+r = [V row | 1.0] (0 if padding)
    Output: out [nblocks*65, W] f32: block j rows [65j:65j+65], cols [:w_j] =
      [ O^T numerator (64 rows) ; denominator (1 row) ] for n-cols j*W..+w_j.
    """
    key = (ncap, mcap)
    if key in _build_cache:
        return _build_cache[key]

    import concourse.bacc as bacc
    import concourse.mybir as mybir
    import concourse.tile as tile

    f32 = mybir.dt.float32
    bf16 = mybir.dt.bfloat16
    i16 = mybir.dt.int16
    mchunks = mcap // 128
    npairs = (mchunks + 1) // 2
    blocks = _blocks(ncap)
    nblocks = len(blocks)
    EXP = mybir.ActivationFunctionType.Exp

    # Schraudolph fast exp on bf16 bit pattern, via int16:
    #   i16 = (int16)(s * (2^7/ln2)/8 + (127*2^7 - C)) ; bitcast -> bf16
    FEXP_A = float(2 ** 7 / np.log(2.0) / 8.0)
    FEXP_B = float(127 * 2 ** 7 - 7.5)

    nc = bacc.Bacc("TRN2", target_bir_lowering=False, debug=False,
                   num_devices=N_CORES)
    qt_d = nc.dram_tensor("qt", [64, ncap], bf16, kind="ExternalInput")
    ktp_d = nc.dram_tensor("ktp", [128, npairs * 128], bf16,
                           kind="ExternalInput")
    vext_d = nc.dram_tensor("vext", [128, mchunks * 65], bf16,
                            kind="ExternalInput")
    out_d = nc.dram_tensor("out", [nblocks * 65, W], f32,
                           kind="ExternalOutput")

    with tile.TileContext(nc) as tc:
        with (
            tc.tile_pool(name="resident", bufs=1) as resident,
            tc.tile_pool(name="pt", bufs=8) as ptp,
            tc.tile_pool(name="osb", bufs=2) as osbp,
            tc.tile_pool(name="psum_st", bufs=4, space="PSUM") as psum_st,
            tc.tile_pool(name="psum_olo", bufs=2, space="PSUM") as psum_olo,
            tc.tile_pool(name="psum_ohi", bufs=2, space="PSUM") as psum_ohi,
        ):
            # input DMAs: small head slices first so compute starts early;
            # configs spread over idle sequencers so they don't serialize
            kt_sb = resident.tile([128, npairs * 128], bf16)
            kcut = min(384, npairs * 128)
            nc.sync.dma_start(kt_sb[:, 0:kcut], ktp_d[:, 0:kcut])
            w0 = blocks[0][1]
            qt_sb = resident.tile([128, ncap], bf16)
            nc.gpsimd.dma_start(qt_sb[0:64, 0:w0], qt_d[:, 0:w0])
            nc.gpsimd.dma_start(qt_sb[64:128, 0:w0], qt_d[:, 0:w0])
            v_sb = resident.tile([128, mchunks * 65], bf16)
            vcut = min(130, mchunks * 65)
            nc.sync.dma_start(v_sb[:, 0:vcut], vext_d[:, 0:vcut])
            if kcut < npairs * 128:
                nc.sync.dma_start(kt_sb[:, kcut:], ktp_d[:, kcut:])
            if vcut < mchunks * 65:
                nc.sync.dma_start(v_sb[:, vcut:], vext_d[:, vcut:])
            if ncap > w0:
                nc.gpsimd.dma_start(qt_sb[0:64, w0:], qt_d[:, w0:])
                nc.gpsimd.dma_start(qt_sb[64:128, w0:], qt_d[:, w0:])

            # PE clock warm-up: the tensor engine p-state ramps only under
            # sustained load, and the PE would otherwise idle ~2us waiting
            # for the input DMAs. Chain small dummy matmuls on a zeroed
            # scratch tile so the first real S matmuls run at full clock.
            warm = resident.tile([64, 64], bf16)
            nc.vector.memset(warm[:, :], 0.0)
            wps = psum_st.tile([128, 512], f32, tag="st")
            for _ in range(14):
                nc.tensor.matmul(wps[0:64, 0:64], warm[:, :], warm[:, :],
                                 start=True, stop=True, tile_position=(0, 0),
                                 skip_group_check=True)

            def drain(jb, olo, ohi, w):
                # fold the ohi row-group accumulator into olo: ScalarE
                # copies ohi out of PSUM, VectorE adds, then DMA
                ohi_sb = osbp.tile([65, w], f32, tag="ohi_sb")
                osb = osbp.tile([65, w], f32, tag="osb")
                nc.scalar.copy(ohi_sb[:, :], ohi[:, :])
                nc.vector.tensor_add(osb[:, :], olo[:, :], ohi_sb[:, :])
                nc.gpsimd.dma_start(out_d[jb * 65:(jb + 1) * 65, 0:w],
                                    osb[:, :])

            # flat pipeline over (block, pair-slot) with S-matmul lookahead
            # LA=1 matched to the st pool depth (bufs=4 single-bank tiles =
            # 2 slots): on the in-order PE queue S(s) sits directly after
            # PV(s-2), and both are gated by the same event (exps(s-2)
            # finishing, which frees s's st buffers) -- so the exp engines
            # run decoupled at full rate. Each slot's two chunks go to the
            # two exp engines concurrently (chunk A -> ScalarE exact, chunk
            # B -> VectorE fast-exp). Every matmul in the kernel has
            # tile_size (64, 128), so the PE never pays a reconfiguration
            # bubble: PV contracts k=64 per row group, with chunk rows 0-63
            # accumulating into olo and rows 64-127 into ohi.
            slots = [(jb, j0, w, p) for jb, (j0, w) in enumerate(blocks)
                     for p in range(npairs)]
            LA = 1
            olos = {}
            sts = {}

            def s_mm(idx):
                jb, j0, w, p = slots[idx]
                is_pair = (2 * p + 1 < mchunks)
                # S^T for the two chunks of pair p: k=64 row groups 0-63 /
                # 64-127 co-stream on the PE
                st_a = psum_st.tile([128, w], f32, tag="st")
                nc.tensor.matmul(
                    st_a[:, :], kt_sb[0:64, p * 128:(p + 1) * 128],
                    qt_sb[0:64, j0:j0 + w],
                    start=True, stop=True, tile_position=(0, 0),
                    skip_group_check=True)
                st_b = None
                if is_pair:
                    st_b = psum_st.tile([128, w], f32, tag="st")
                    nc.tensor.matmul(
                        st_b[:, :], kt_sb[64:128, p * 128:(p + 1) * 128],
                        qt_sb[64:128, j0:j0 + w],
                        start=True, stop=True, tile_position=(64, 0),
                        skip_group_check=True)
                sts[idx] = (st_a, st_b)

            def pv(idx, pts):
                jb, j0, w, p = slots[idx]
                mi0 = 2 * p
                if mi0 == 0:
                    olo = psum_olo.tile([65, w], f32, tag="olo")
                    ohi = psum_ohi.tile([65, w], f32, tag="ohi")
                    olos[jb] = (olo, ohi)
                olo, ohi = olos[jb]
                last = False
                for mi, pt in ((mi0, pts[0]), (mi0 + 1, pts[1])):
                    if pt is None:
                        continue
                    last = (mi == mchunks - 1)
                    ap = pt[:, :].bitcast(bf16) if pt.dtype == i16 \
                        else pt[:, :]
                    nc.tensor.matmul(
                        olo[:, :], v_sb[0:64, mi * 65:(mi + 1) * 65],
                        ap[0:64, :], start=(mi == 0), stop=last,
                        tile_position=(0, 0), skip_group_check=True)
                    nc.tensor.matmul(
                        ohi[:, :], v_sb[64:128, mi * 65:(mi + 1) * 65],
                        ap[64:128, :], start=(mi == 0), stop=last,
                        tile_position=(64, 0), skip_group_check=True)
                if last:
                    drain(jb, olo, ohi, w)

            prev_pts = None
            for idx in range(len(slots)):
                jb, j0, w, p = slots[idx]
                if idx == 0:
                    s_mm(0)
                if idx + LA < len(slots):
                    s_mm(idx + LA)
                if prev_pts is not None:
                    pv(idx - 1, prev_pts)
                # P^T = exp(S^T/8): BOTH chunks of a slot go to one engine
                # (even slots ScalarE exact exp, odd slots VectorE fast-exp)
                # so each PV matmul waits on a single engine and a hiccup on
                # one engine (e.g. a drain copy) doesn't convoy the other.
                st_a, st_b = sts.pop(idx)
                on_act = (p % 2 == 0)
                if on_act:
                    pt_a = ptp.tile([128, w], bf16, tag="pt")
                    nc.scalar.activation(pt_a[:, :], st_a[:, :], EXP,
                                         scale=0.125)
                else:
                    pt_a = ptp.tile([128, w], i16, tag="pt")
                    nc.vector.tensor_scalar(
                        pt_a[:, :], st_a[:, :], FEXP_A, FEXP_B,
                        mybir.AluOpType.mult, mybir.AluOpType.add)
                pt_b = None
                if st_b is not None:
                    if on_act:
                        pt_b = ptp.tile([128, w], bf16, tag="pt")
                        nc.scalar.activation(pt_b[:, :], st_b[:, :], EXP,
                                             scale=0.125)
                    else:
                        pt_b = ptp.tile([128, w], i16, tag="pt")
                        nc.vector.tensor_scalar(
                            pt_b[:, :], st_b[:, :], FEXP_A, FEXP_B,
                            mybir.AluOpType.mult, mybir.AluOpType.add)
                prev_pts = (pt_a, pt_b)
            pv(len(slots) - 1, prev_pts)

    nc.compile()
    _build_cache[key] = nc
    return nc


def _run(inputs, trace=False):
    queries = np.asarray(inputs["queries"], dtype=np.float32)
    keys = np.asarray(inputs["keys"], dtype=np.float32)
    values = np.asarray(inputs["values"], dtype=np.float32)
    mask_query = np.asarray(inputs["mask_query"])
    mask_key = np.asarray(inputs["mask_key"])

    b, n, d = queries.shape
    dv = values.shape[2]
    assert b == N_CORES, f"batch {b} != {N_CORES} cores"
    bf = ml_dtypes.bfloat16

    idx_q = [np.flatnonzero(mask_query[i]) for i in range(b)]
    idx_k = [np.flatnonzero(mask_key[i]) for i in range(b)]
    ncap = max(max(len(ix) for ix in idx_q), 64)
    mcap = ((max(max(len(ix) for ix in idx_k), 1) + 127) // 128) * 128
    mchunks = mcap // 128
    npairs = (mchunks + 1) // 2
    blocks = _blocks(ncap)
    nblocks = len(blocks)

    qt = np.zeros((b, 64, ncap), bf)
    ktp = np.zeros((b, 128, npairs * 128), bf)
    vext = np.zeros((b, 128, mchunks * 65), bf)
    for i in range(b):
        nq, nk = len(idx_q[i]), len(idx_k[i])
        qt[i, :, :nq] = queries[i, idx_q[i]].T.astype(bf)
        kc_t = np.zeros((64, mcap), np.float32)
        kc_t[:, :nk] = keys[i, idx_k[i]].T
        kc_t = kc_t.astype(bf)
        for p in range(npairs):
            ktp[i, 0:64, p * 128:(p + 1) * 128] = \
                kc_t[:, (2 * p) * 128:(2 * p + 1) * 128]
            if 2 * p + 1 < mchunks:
                ktp[i, 64:128, p * 128:(p + 1) * 128] = \
                    kc_t[:, (2 * p + 1) * 128:(2 * p + 2) * 128]
        ve = np.zeros((mcap, 65), np.float32)
        ve[:nk, :dv] = values[i, idx_k[i]]
        ve[:nk, dv] = 1.0
        # partition-major: [chunk, row] -> [row_in_chunk, chunk*65+col]
        vext[i] = ve.reshape(mchunks, 128, 65).transpose(1, 0, 2) \
                    .reshape(128, mchunks * 65).astype(bf)

    nc = _build(ncap, mcap)

    from concourse.bass_utils import run_bass_kernel_spmd
    in_maps = [{"qt": qt[i], "ktp": ktp[i], "vext": vext[i]} for i in range(b)]
    res = run_bass_kernel_spmd(nc, in_maps, core_ids=list(range(N_CORES)),
                               trace=trace)

    out = np.empty((b, n, dv), np.float32)
    for i in range(b):
        ot = res.results[i]["out"]  # [nblocks*65, W]
        nq, nk = len(idx_q[i]), len(idx_k[i])
        full = np.empty((65, ncap), np.float32)
        for jb, (j0, wd) in enumerate(blocks):
            full[:, j0:j0 + wd] = ot[jb * 65:(jb + 1) * 65, :wd]
        num = full[:dv, :nq]
        den = full[dv, :nq]
        if nk > 0:
            out[i, :, :] = values[i, idx_k[i]].mean(axis=0)
        else:
            out[i, :, :] = 0.0
        if nq > 0:
            out[i, idx_q[i], :] = (num / den).T
    return out, res


def kernel(**inputs):
    out, _ = _run(inputs, trace=False)
    return out


# revision 30
# speedup vs baseline: 1.0315x; 1.0315x over previous
"""Sparse masked attention on 8 TRN2 NeuronCores.

reference:  O = softmax((Q K^T * mq[:,None] + log(mk[None,:])) / 8) @ V
  - rows with mq=0: scores all equal -> uniform average of V over mk=1 keys
  - keys with mk=0: exactly dropped from the softmax

Strategy: batch (b=8) is data-parallel across the 8 cores. On the host we
compact each batch to its unmasked queries/keys (~n/2 each), so each core
computes a dense ~2176x2176 attention instead of 4096x4096:

  S^T[mchunk, n] = (Kc^T chunk).T @ Qc^T      (TensorE bf16, d=64, row-group
                                               pairs: two chunks co-stream)
  P^T = exp(S^T / 8) -> bf16                  (ScalarE exact exp on chunk A,
                                               VectorE fast-exp on chunk B,
                                               concurrently each slot)
  O^T[65, n]    += Vext[mchunk].T @ P^T       (TensorE bf16, k=64 row groups
                                               into olo/ohi PSUM accumulators)

where Vext = [V | 1]: the ones column accumulates the softmax denominator.
No row-max subtraction is needed: scores/8 ~ N(0,1), exp stays in range.
Every matmul keeps PE tile_size (64,128) so the array never pays a
reconfiguration bubble; S matmuls are issued one pair-slot ahead of the
exp-gated PV matmuls so the two exp engines never serialize against the
in-order PE queue. The host divides by the denominator, scatters rows
back, and fills masked query rows with mean(V[mk=1]).
"""

import numpy as np
import ml_dtypes

N_CORES = 8
W = 512  # n-block width (PSUM bank / fp32-accum matmul free-dim limit)


_build_cache = {}


def _blocks(ncap):
    """n-blocks as (col_offset, width); the remainder block runs last (it
    needs all K/V chunks, which are still streaming in at kernel start)."""
    blks = [(i * W, W) for i in range(ncap // W)]
    if ncap % W:
        blks.append((ncap - ncap % W, ncap % W))
    return blks


def _build(ncap, mcap):
    """Per-core graph. Inputs (per core):
      qt   [64, ncap]            bf16   Q^T (compacted, padded)
      ktp  [128, npairs*128]     bf16   K^T chunk pairs: pair p = chunk 2p on
                                        partitions 0-63, chunk 2p+1 on 64-127
      vext [128, mchunks*65]     bf16   partition-major Vext chunks: partition
                                        r, cols [c*65:(c+1)*65] = Vext row
                                        c*128+r = [V row | 1.0] (0 if padding)
    Output: out [nblocks*65, W] f32: block j rows [65j:65j+65], cols [:w_j] =
      [ O^T numerator (64 rows) ; denominator (1 row) ] for n-cols j*W..+w_j.
    """
    key = (ncap, mcap)
    if key in _build_cache:
        return _build_cache[key]

    import concourse.bacc as bacc
    import concourse.mybir as mybir
    import concourse.tile as tile

    f32 = mybir.dt.float32
    bf16 = mybir.dt.bfloat16
    i16 = mybir.dt.int16
    mchunks = mcap // 128
    npairs = (mchunks + 1) // 2
    blocks = _blocks(ncap)
    nblocks = len(blocks)
    EXP = mybir.ActivationFunctionType.Exp

    # Schraudolph fast exp on bf16 bit pattern, via int16:
    #   i16 = (int16)(s * (2^7/ln2)/8 + (127*2^7 - C)) ; bitcast -> bf16
    FEXP_A = float(2 ** 7 / np.log(2.0) / 8.0)
    FEXP_B = float(127 * 2 ** 7 - 7.5)

    nc = bacc.Bacc("TRN2", target_bir_lowering=False, debug=False,
                   num_devices=N_CORES)
    qt_d = nc.dram_tensor("qt", [64, ncap], bf16, kind="ExternalInput")
    ktp_d = nc.dram_tensor("ktp", [128, npairs * 128], bf16,
                           kind="ExternalInput")
    vext_d = nc.dram_tensor("vext", [128, mchunks * 65], bf16,
                            kind="ExternalInput")
    out_d = nc.dram_tensor("out", [nblocks * 65, W], f32,
                           kind="ExternalOutput")

    with tile.TileContext(nc) as tc:
        with (
            tc.tile_pool(name="resident", bufs=1) as resident,
            tc.tile_pool(name="pt", bufs=8) as ptp,
            tc.tile_pool(name="osb", bufs=2) as osbp,
            tc.tile_pool(name="psum_st", bufs=4, space="PSUM") as psum_st,
            tc.tile_pool(name="psum_olo", bufs=2, space="PSUM") as psum_olo,
            tc.tile_pool(name="psum_ohi", bufs=2, space="PSUM") as psum_ohi,
        ):
            # input DMAs: small head slices first so compute starts early;
            # configs spread over idle sequencers so they don't serialize
            kt_sb = resident.tile([128, npairs * 128], bf16)
            kcut = min(384, npairs * 128)
            nc.sync.dma_start(kt_sb[:, 0:kcut], ktp_d[:, 0:kcut])
            w0 = blocks[0][1]
            qt_sb = resident.tile([128, ncap], bf16)
            nc.gpsimd.dma_start(qt_sb[0:64, 0:w0], qt_d[:, 0:w0])
            nc.gpsimd.dma_start(qt_sb[64:128, 0:w0], qt_d[:, 0:w0])
            v_sb = resident.tile([128, mchunks * 65], bf16)
            vcut = min(130, mchunks * 65)
            nc.sync.dma_start(v_sb[:, 0:vcut], vext_d[:, 0:vcut])
            if kcut < npairs * 128:
                nc.sync.dma_start(kt_sb[:, kcut:], ktp_d[:, kcut:])
            if vcut < mchunks * 65:
                nc.sync.dma_start(v_sb[:, vcut:], vext_d[:, vcut:])
            if ncap > w0:
                nc.gpsimd.dma_start(qt_sb[0:64, w0:], qt_d[:, w0:])
                nc.gpsimd.dma_start(qt_sb[64:128, w0:], qt_d[:, w0:])

            # PE clock warm-up: the tensor engine p-state ramps only under
            # sustained load, and the PE would otherwise idle ~2us waiting
            # for the input DMAs. Chain small dummy matmuls on a zeroed
            # scratch tile so the first real S matmuls run at full clock.
            warm = resident.tile([64, 64], bf16)
            nc.vector.memset(warm[:, :], 0.0)
            wps = psum_st.tile([128, 512], f32, tag="st")
            for _ in range(14):
                nc.tensor.matmul(wps[0:64, 0:64], warm[:, :], warm[:, :],
                                 start=True, stop=True, tile_position=(0, 0),
                                 skip_group_check=True)

            def drain(jb, olo, ohi, w):
                # fold the ohi row-group accumulator into olo: ScalarE
                # copies ohi out of PSUM, VectorE adds, then DMA
                ohi_sb = osbp.tile([65, w], f32, tag="ohi_sb")
                osb = osbp.tile([65, w], f32, tag="osb")
                nc.scalar.copy(ohi_sb[:, :], ohi[:, :])
                nc.vector.tensor_add(osb[:, :], olo[:, :], ohi_sb[:, :])
                nc.gpsimd.dma_start(out_d[jb * 65:(jb + 1) * 65, 0:w],
                                    osb[:, :])

            # flat pipeline over (block, pair-slot) with S-matmul lookahead
            # LA=1 matched to the st pool depth (bufs=4 single-bank tiles =
            # 2 slots): on the in-order PE queue S(s) sits directly after
            # PV(s-2), and both are gated by the same event (exps(s-2)
            # finishing, which frees s's st buffers) -- so the exp engines
            # run decoupled at full rate. Each slot's two chunks go to the
            # two exp engines concurrently (chunk A -> ScalarE exact, chunk
            # B -> VectorE fast-exp). Every matmul in the kernel has
            # tile_size (64, 128), so the PE never pays a reconfiguration
            # bubble: PV contracts k=64 per row group, with chunk rows 0-63
            # accumulating into olo and rows 64-127 into ohi.
            slots = [(jb, j0, w, p) for jb, (j0, w) in enumerate(blocks)
                     for p in range(npairs)]
            LA = 1
            olos = {}
            sts = {}

            def s_mm(idx):
                jb, j0, w, p = slots[idx]
                is_pair = (2 * p + 1 < mchunks)
                # S^T for the two chunks of pair p: k=64 row groups 0-63 /
                # 64-127 co-stream on the PE
                st_a = psum_st.tile([128, w], f32, tag="st")
                nc.tensor.matmul(
                    st_a[:, :], kt_sb[0:64, p * 128:(p + 1) * 128],
                    qt_sb[0:64, j0:j0 + w],
                    start=True, stop=True, tile_position=(0, 0),
                    skip_group_check=True)
                st_b = None
                if is_pair:
                    st_b = psum_st.tile([128, w], f32, tag="st")
                    nc.tensor.matmul(
                        st_b[:, :], kt_sb[64:128, p * 128:(p + 1) * 128],
                        qt_sb[64:128, j0:j0 + w],
                        start=True, stop=True, tile_position=(64, 0),
                        skip_group_check=True)
                sts[idx] = (st_a, st_b)

            def pv(idx, pts):
                jb, j0, w, p = slots[idx]
                mi0 = 2 * p
                if mi0 == 0:
                    olo = psum_olo.tile([65, w], f32, tag="olo")
                    ohi = psum_ohi.tile([65, w], f32, tag="ohi")
                    olos[jb] = (olo, ohi)
                olo, ohi = olos[jb]
                last = False
                for mi, pt in ((mi0, pts[0]), (mi0 + 1, pts[1])):
                    if pt is None:
                        continue
                    last = (mi == mchunks - 1)
                    ap = pt[:, :].bitcast(bf16) if pt.dtype == i16 \
                        else pt[:, :]
                    nc.tensor.matmul(
                        olo[:, :], v_sb[0:64, mi * 65:(mi + 1) * 65],
                        ap[0:64, :], start=(mi == 0), stop=last,
                        tile_position=(0, 0), skip_group_check=True)
                    nc.tensor.matmul(
                        ohi[:, :], v_sb[64:128, mi * 65:(mi + 1) * 65],
                        ap[64:128, :], start=(mi == 0), stop=last,
                        tile_position=(64, 0), skip_group_check=True)
                if last:
                    drain(jb, olo, ohi, w)

            prev_pts = None
            for idx in range(len(slots)):
                jb, j0, w, p = slots[idx]
                if idx == 0:
                    s_mm(0)
                if idx + LA < len(slots):
                    s_mm(idx + LA)
                if prev_pts is not None:
                    pv(idx - 1, prev_pts)
                # P^T = exp(S^T/8): chunk A exact on ScalarE, chunk B
                # fast-exp on VectorE, concurrently each slot
                st_a, st_b = sts.pop(idx)
                pt_a = ptp.tile([128, w], bf16, tag="pt")
                nc.scalar.activation(pt_a[:, :], st_a[:, :], EXP,
                                     scale=0.125)
                pt_b = None
                if st_b is not None:
                    pt_b = ptp.tile([128, w], i16, tag="pt")
                    nc.vector.tensor_scalar(
                        pt_b[:, :], st_b[:, :], FEXP_A, FEXP_B,
                        mybir.AluOpType.mult, mybir.AluOpType.add)
                prev_pts = (pt_a, pt_b)
            pv(len(slots) - 1, prev_pts)

    nc.compile()
    _build_cache[key] = nc
    return nc


def _run(inputs, trace=False):
    queries = np.asarray(inputs["queries"], dtype=np.float32)
    keys = np.asarray(inputs["keys"], dtype=np.float32)
    values = np.asarray(inputs["values"], dtype=np.float32)
    mask_query = np.asarray(inputs["mask_query"])
    mask_key = np.asarray(inputs["mask_key"])

    b, n, d = queries.shape
    dv = values.shape[2]
    assert b == N_CORES, f"batch {b} != {N_CORES} cores"
    bf = ml_dtypes.bfloat16

    idx_q = [np.flatnonzero(mask_query[i]) for i in range(b)]
    idx_k = [np.flatnonzero(mask_key[i]) for i in range(b)]
    ncap = max(max(len(ix) for ix in idx_q), 64)
    mcap = ((max(max(len(ix) for ix in idx_k), 1) + 127) // 128) * 128
    mchunks = mcap // 128
    npairs = (mchunks + 1) // 2
    blocks = _blocks(ncap)
    nblocks = len(blocks)

    qt = np.zeros((b, 64, ncap), bf)
    ktp = np.zeros((b, 128, npairs * 128), bf)
    vext = np.zeros((b, 128, mchunks * 65), bf)
    for i in range(b):
        nq, nk = len(idx_q[i]), len(idx_k[i])
        qt[i, :, :nq] = queries[i, idx_q[i]].T.astype(bf)
        kc_t = np.zeros((64, mcap), np.float32)
        kc_t[:, :nk] = keys[i, idx_k[i]].T
        kc_t = kc_t.astype(bf)
        for p in range(npairs):
            ktp[i, 0:64, p * 128:(p + 1) * 128] = \
                kc_t[:, (2 * p) * 128:(2 * p + 1) * 128]
            if 2 * p + 1 < mchunks:
                ktp[i, 64:128, p * 128:(p + 1) * 128] = \
                    kc_t[:, (2 * p + 1) * 128:(2 * p + 2) * 128]
        ve = np.zeros((mcap, 65), np.float32)
        ve[:nk, :dv] = values[i, idx_k[i]]
        ve[:nk, dv] = 1.0
        # partition-major: [chunk, row] -> [row_in_chunk, chunk*65+col]
        vext[i] = ve.reshape(mchunks, 128, 65).transpose(1, 0, 2) \
                    .reshape(128, mchunks * 65).astype(bf)

    nc = _build(ncap, mcap)

    from concourse.bass_utils import run_bass_kernel_spmd
    in_maps = [{"qt": qt[i], "ktp": ktp[i], "vext": vext[i]} for i in range(b)]
    res = run_bass_kernel_spmd(nc, in_maps, core_ids=list(range(N_CORES)),
                               trace=trace)

    out = np.empty((b, n, dv), np.float32)
    for i in range(b):
        ot = res.results[i]["out"]  # [nblocks*65, W]
        nq, nk = len(idx_q[i]), len(idx_k[i])
        full = np.empty((65, ncap), np.float32)
        for jb, (j0, wd) in enumerate(blocks):
            full[:, j0:j0 + wd] = ot[jb * 65:(jb + 1) * 65, :wd]
        num = full[:dv, :nq]
        den = full[dv, :nq]
        if nk > 0:
            out[i, :, :] = values[i, idx_k[i]].mean(axis=0)
        else:
            out[i, :, :] = 0.0
        if nq > 0:
            out[i, idx_q[i], :] = (num / den).T
    return out, res


def kernel(**inputs):
    out, _ = _run(inputs, trace=False)
    return out


# revision 31
# speedup vs baseline: 1.0536x; 1.0214x over previous
"""Sparse masked attention on 8 TRN2 NeuronCores.

reference:  O = softmax((Q K^T * mq[:,None] + log(mk[None,:])) / 8) @ V
  - rows with mq=0: scores all equal -> uniform average of V over mk=1 keys
  - keys with mk=0: exactly dropped from the softmax

Strategy: batch (b=8) is data-parallel across the 8 cores. On the host we
compact each batch to its unmasked queries/keys (~n/2 each), so each core
computes a dense ~2176x2176 attention instead of 4096x4096:

  S^T[mchunk, n] = (Kc^T chunk).T @ Qc^T      (TensorE bf16, d=64, row-group
                                               pairs: two chunks co-stream)
  P^T = exp(S^T / 8) -> bf16                  (ScalarE exact exp on chunk A,
                                               VectorE fast-exp on chunk B,
                                               concurrently each slot)
  O^T[65, n]    += Vext[mchunk].T @ P^T       (TensorE bf16, k=64 row groups
                                               into olo/ohi PSUM accumulators)

where Vext = [V | 1]: the ones column accumulates the softmax denominator.
No row-max subtraction is needed: scores/8 ~ N(0,1), exp stays in range.
Every matmul keeps PE tile_size (64,128) so the array never pays a
reconfiguration bubble; S matmuls are issued one pair-slot ahead of the
exp-gated PV matmuls so the two exp engines never serialize against the
in-order PE queue. The host divides by the denominator, scatters rows
back, and fills masked query rows with mean(V[mk=1]).
"""

import numpy as np
import ml_dtypes

N_CORES = 8
W = 512  # n-block width (PSUM bank / fp32-accum matmul free-dim limit)


_build_cache = {}


def _blocks(ncap):
    """n-blocks as (col_offset, width); the remainder block runs last (it
    needs all K/V chunks, which are still streaming in at kernel start)."""
    blks = [(i * W, W) for i in range(ncap // W)]
    if ncap % W:
        blks.append((ncap - ncap % W, ncap % W))
    return blks


def _build(ncap, mcap):
    """Per-core graph. Inputs (per core):
      qt   [64, ncap]            bf16   Q^T (compacted, padded)
      ktp  [128, npairs*128]     bf16   K^T chunk pairs: pair p = chunk 2p on
                                        partitions 0-63, chunk 2p+1 on 64-127
      vext [128, mchunks*65]     bf16   partition-major Vext chunks: partition
                                        r, cols [c*65:(c+1)*65] = Vext row
                                        c*128+r = [V row | 1.0] (0 if padding)
    Output: out [nblocks*65, W] f32: block j rows [65j:65j+65], cols [:w_j] =
      [ O^T numerator (64 rows) ; denominator (1 row) ] for n-cols j*W..+w_j.
    """
    key = (ncap, mcap)
    if key in _build_cache:
        return _build_cache[key]

    import concourse.bacc as bacc
    import concourse.mybir as mybir
    import concourse.tile as tile

    f32 = mybir.dt.float32
    bf16 = mybir.dt.bfloat16
    i16 = mybir.dt.int16
    mchunks = mcap // 128
    npairs = (mchunks + 1) // 2
    blocks = _blocks(ncap)
    nblocks = len(blocks)
    EXP = mybir.ActivationFunctionType.Exp

    # Schraudolph fast exp on bf16 bit pattern, via int16:
    #   i16 = (int16)(s * (2^7/ln2)/8 + (127*2^7 - C)) ; bitcast -> bf16
    FEXP_A = float(2 ** 7 / np.log(2.0) / 8.0)
    FEXP_B = float(127 * 2 ** 7 - 7.5)

    nc = bacc.Bacc("TRN2", target_bir_lowering=False, debug=False,
                   num_devices=N_CORES)
    qt_d = nc.dram_tensor("qt", [64, ncap], bf16, kind="ExternalInput")
    ktp_d = nc.dram_tensor("ktp", [128, npairs * 128], bf16,
                           kind="ExternalInput")
    vext_d = nc.dram_tensor("vext", [128, mchunks * 65], bf16,
                            kind="ExternalInput")
    out_d = nc.dram_tensor("out", [nblocks * 65, W], f32,
                           kind="ExternalOutput")

    with tile.TileContext(nc) as tc:
        with (
            tc.tile_pool(name="resident", bufs=1) as resident,
            tc.tile_pool(name="pt", bufs=8) as ptp,
            tc.tile_pool(name="osb", bufs=2) as osbp,
            tc.tile_pool(name="psum_st", bufs=5, space="PSUM") as psum_st,
            tc.tile_pool(name="psum_olo", bufs=2, space="PSUM") as psum_olo,
            tc.tile_pool(name="psum_ohi", bufs=1, space="PSUM") as psum_ohi,
        ):
            # input DMAs: small head slices first so compute starts early;
            # configs spread over idle sequencers so they don't serialize
            kt_sb = resident.tile([128, npairs * 128], bf16)
            kcut = min(384, npairs * 128)
            nc.sync.dma_start(kt_sb[:, 0:kcut], ktp_d[:, 0:kcut])
            w0 = blocks[0][1]
            qt_sb = resident.tile([128, ncap], bf16)
            nc.gpsimd.dma_start(qt_sb[0:64, 0:w0], qt_d[:, 0:w0])
            nc.gpsimd.dma_start(qt_sb[64:128, 0:w0], qt_d[:, 0:w0])
            v_sb = resident.tile([128, mchunks * 65], bf16)
            vcut = min(130, mchunks * 65)
            nc.sync.dma_start(v_sb[:, 0:vcut], vext_d[:, 0:vcut])
            if kcut < npairs * 128:
                nc.sync.dma_start(kt_sb[:, kcut:], ktp_d[:, kcut:])
            if vcut < mchunks * 65:
                nc.sync.dma_start(v_sb[:, vcut:], vext_d[:, vcut:])
            if ncap > w0:
                nc.gpsimd.dma_start(qt_sb[0:64, w0:], qt_d[:, w0:])
                nc.gpsimd.dma_start(qt_sb[64:128, w0:], qt_d[:, w0:])

            # PE clock warm-up: the tensor engine p-state ramps only under
            # sustained load, and the PE would otherwise idle ~2us waiting
            # for the input DMAs. Chain small dummy matmuls on a zeroed
            # scratch tile so the first real S matmuls run at full clock.
            warm = resident.tile([64, 64], bf16)
            nc.vector.memset(warm[:, :], 0.0)
            wps = psum_st.tile([128, 512], f32, tag="st")
            for _ in range(14):
                nc.tensor.matmul(wps[0:64, 0:64], warm[:, :], warm[:, :],
                                 start=True, stop=True, tile_position=(0, 0),
                                 skip_group_check=True)

            def drain(jb, olo, ohi, w):
                # fold the ohi row-group accumulator into olo: ScalarE
                # copies ohi out of PSUM, VectorE adds, then DMA
                ohi_sb = osbp.tile([65, w], f32, tag="ohi_sb")
                osb = osbp.tile([65, w], f32, tag="osb")
                nc.scalar.copy(ohi_sb[:, :], ohi[:, :])
                nc.vector.tensor_add(osb[:, :], olo[:, :], ohi_sb[:, :])
                nc.gpsimd.dma_start(out_d[jb * 65:(jb + 1) * 65, 0:w],
                                    osb[:, :])

            # flat pipeline over (block, pair-slot) with S-matmul lookahead
            # LA=1 matched to the st pool depth (bufs=4 single-bank tiles =
            # 2 slots): on the in-order PE queue S(s) sits directly after
            # PV(s-2), and both are gated by the same event (exps(s-2)
            # finishing, which frees s's st buffers) -- so the exp engines
            # run decoupled at full rate. Each slot's two chunks go to the
            # two exp engines concurrently (chunk A -> ScalarE exact, chunk
            # B -> VectorE fast-exp). Every matmul in the kernel has
            # tile_size (64, 128), so the PE never pays a reconfiguration
            # bubble: PV contracts k=64 per row group, with chunk rows 0-63
            # accumulating into olo and rows 64-127 into ohi.
            slots = [(jb, j0, w, p) for jb, (j0, w) in enumerate(blocks)
                     for p in range(npairs)]
            LA = 1
            olos = {}
            sts = {}

            def s_mm(idx):
                jb, j0, w, p = slots[idx]
                is_pair = (2 * p + 1 < mchunks)
                # S^T for the two chunks of pair p: k=64 row groups 0-63 /
                # 64-127 co-stream on the PE
                st_a = psum_st.tile([128, w], f32, tag="st")
                nc.tensor.matmul(
                    st_a[:, :], kt_sb[0:64, p * 128:(p + 1) * 128],
                    qt_sb[0:64, j0:j0 + w],
                    start=True, stop=True, tile_position=(0, 0),
                    skip_group_check=True)
                st_b = None
                if is_pair:
                    st_b = psum_st.tile([128, w], f32, tag="st")
                    nc.tensor.matmul(
                        st_b[:, :], kt_sb[64:128, p * 128:(p + 1) * 128],
                        qt_sb[64:128, j0:j0 + w],
                        start=True, stop=True, tile_position=(64, 0),
                        skip_group_check=True)
                sts[idx] = (st_a, st_b)

            def pv(idx, pts):
                jb, j0, w, p = slots[idx]
                mi0 = 2 * p
                if mi0 == 0:
                    olo = psum_olo.tile([65, w], f32, tag="olo")
                    ohi = psum_ohi.tile([65, w], f32, tag="ohi")
                    olos[jb] = (olo, ohi)
                olo, ohi = olos[jb]
                last = False
                for mi, pt in ((mi0, pts[0]), (mi0 + 1, pts[1])):
                    if pt is None:
                        continue
                    last = (mi == mchunks - 1)
                    ap = pt[:, :].bitcast(bf16) if pt.dtype == i16 \
                        else pt[:, :]
                    nc.tensor.matmul(
                        olo[:, :], v_sb[0:64, mi * 65:(mi + 1) * 65],
                        ap[0:64, :], start=(mi == 0), stop=last,
                        tile_position=(0, 0), skip_group_check=True)
                    nc.tensor.matmul(
                        ohi[:, :], v_sb[64:128, mi * 65:(mi + 1) * 65],
                        ap[64:128, :], start=(mi == 0), stop=last,
                        tile_position=(64, 0), skip_group_check=True)
                if last:
                    drain(jb, olo, ohi, w)

            prev_pts = None
            for idx in range(len(slots)):
                jb, j0, w, p = slots[idx]
                if idx == 0:
                    s_mm(0)
                if idx + LA < len(slots):
                    s_mm(idx + LA)
                if prev_pts is not None:
                    pv(idx - 1, prev_pts)
                # P^T = exp(S^T/8): chunk A exact on ScalarE, chunk B
                # fast-exp on VectorE, concurrently each slot
                st_a, st_b = sts.pop(idx)
                pt_a = ptp.tile([128, w], bf16, tag="pt")
                nc.scalar.activation(pt_a[:, :], st_a[:, :], EXP,
                                     scale=0.125)
                pt_b = None
                if st_b is not None:
                    pt_b = ptp.tile([128, w], i16, tag="pt")
                    nc.vector.tensor_scalar(
                        pt_b[:, :], st_b[:, :], FEXP_A, FEXP_B,
                        mybir.AluOpType.mult, mybir.AluOpType.add)
                prev_pts = (pt_a, pt_b)
            pv(len(slots) - 1, prev_pts)

    nc.compile()
    _build_cache[key] = nc
    return nc


def _run(inputs, trace=False):
    queries = np.asarray(inputs["queries"], dtype=np.float32)
    keys = np.asarray(inputs["keys"], dtype=np.float32)
    values = np.asarray(inputs["values"], dtype=np.float32)
    mask_query = np.asarray(inputs["mask_query"])
    mask_key = np.asarray(inputs["mask_key"])

    b, n, d = queries.shape
    dv = values.shape[2]
    assert b == N_CORES, f"batch {b} != {N_CORES} cores"
    bf = ml_dtypes.bfloat16

    idx_q = [np.flatnonzero(mask_query[i]) for i in range(b)]
    idx_k = [np.flatnonzero(mask_key[i]) for i in range(b)]
    ncap = max(max(len(ix) for ix in idx_q), 64)
    mcap = ((max(max(len(ix) for ix in idx_k), 1) + 127) // 128) * 128
    mchunks = mcap // 128
    npairs = (mchunks + 1) // 2
    blocks = _blocks(ncap)
    nblocks = len(blocks)

    qt = np.zeros((b, 64, ncap), bf)
    ktp = np.zeros((b, 128, npairs * 128), bf)
    vext = np.zeros((b, 128, mchunks * 65), bf)
    for i in range(b):
        nq, nk = len(idx_q[i]), len(idx_k[i])
        qt[i, :, :nq] = queries[i, idx_q[i]].T.astype(bf)
        kc_t = np.zeros((64, mcap), np.float32)
        kc_t[:, :nk] = keys[i, idx_k[i]].T
        kc_t = kc_t.astype(bf)
        for p in range(npairs):
            ktp[i, 0:64, p * 128:(p + 1) * 128] = \
                kc_t[:, (2 * p) * 128:(2 * p + 1) * 128]
            if 2 * p + 1 < mchunks:
                ktp[i, 64:128, p * 128:(p + 1) * 128] = \
                    kc_t[:, (2 * p + 1) * 128:(2 * p + 2) * 128]
        ve = np.zeros((mcap, 65), np.float32)
        ve[:nk, :dv] = values[i, idx_k[i]]
        ve[:nk, dv] = 1.0
        # partition-major: [chunk, row] -> [row_in_chunk, chunk*65+col]
        vext[i] = ve.reshape(mchunks, 128, 65).transpose(1, 0, 2) \
                    .reshape(128, mchunks * 65).astype(bf)

    nc = _build(ncap, mcap)

    from concourse.bass_utils import run_bass_kernel_spmd
    in_maps = [{"qt": qt[i], "ktp": ktp[i], "vext": vext[i]} for i in range(b)]
    res = run_bass_kernel_spmd(nc, in_maps, core_ids=list(range(N_CORES)),
                               trace=trace)

    out = np.empty((b, n, dv), np.float32)
    for i in range(b):
        ot = res.results[i]["out"]  # [nblocks*65, W]
        nq, nk = len(idx_q[i]), len(idx_k[i])
        full = np.empty((65, ncap), np.float32)
        for jb, (j0, wd) in enumerate(blocks):
            full[:, j0:j0 + wd] = ot[jb * 65:(jb + 1) * 65, :wd]
        num = full[:dv, :nq]
        den = full[dv, :nq]
        if nk > 0:
            out[i, :, :] = values[i, idx_k[i]].mean(axis=0)
        else:
            out[i, :, :] = 0.0
        if nq > 0:
            out[i, idx_q[i], :] = (num / den).T
    return out, res


def kernel(**inputs):
    out, _ = _run(inputs, trace=False)
    return out
